# revision 21
# baseline (speedup 1.0000x reference)
"""Trainium2 Bass kernel for nn_Mask_58351425683882.

Computes out = (x * mask) @ from_to with
  x:      [16, 8192]  f32
  mask:   [8192]      f32 (0/1)
  from_to:[8192,8192] f32 (one-hot permutation columns)

Strategy: column-shard from_to across 8 NeuronCores ([8192, 1024] per
core), replicate x/mask. Each core streams its 32MB from_to shard from
HBM (the memory-roofline term) and accumulates the [16, 1024] output
slice on TensorE with x_masked^T as the stationary operand. Host
concatenates the 8 output slices.

Written in raw Bass (explicit engine blocks + semaphores): the Tile
scheduler attaches multi-semaphore waits to DMA/matmul instructions,
which this walrus build rejects ("Too many sync wait commands" — the
HWDGE/LW instruction encodings carry at most one). Raw standalone
wait_ge instructions sidestep that entirely.
"""

import sys

for _p in ("/opt/trn_rl_repo",):
    if _p not in sys.path:
        sys.path.insert(0, _p)

import numpy as np

import concourse.bass as bass
import concourse.mybir as mybir
from concourse.bass_utils import run_bass_kernel_spmd

B = 16          # batch rows of x
N = 8192        # feature dim
NCORES = 8
NSH = N // NCORES       # 1024 output columns per core
P = 128                 # SBUF partitions
KT = N // P             # 64 contraction tiles
NJ = NSH // 512         # 2 PSUM column chunks per core
FTB = 16                # ft streaming buffer depth (ring of SBUF slots)

_F32 = mybir.dt.float32
_F32R = mybir.dt.float32r


def build_nc():
    nc = bass.Bass()

    # xin packs x^T and mask:
    #   cols [0, KT*B):    xin[p, k*B + b] = x[b, k*128 + p]
    #   cols [KT*B, +KT):  xin[p, KT*B + k] = mask[k*128 + p]
    xin = nc.dram_tensor("xin", [P, KT * B + KT], _F32R, kind="ExternalInput")
    # This core's column shard of from_to.
    ft = nc.dram_tensor("ft", [N, NSH], _F32R, kind="ExternalInput")
    out = nc.dram_tensor("out", [B, NSH], _F32, kind="ExternalOutput")

    from contextlib import ExitStack

    with ExitStack() as ctx:
        x_sem = ctx.enter_context(nc.semaphore("x_sem"))
        # One semaphore per ring slot: slot s is reused only after the PE
        # consumed the previous tile in it (pe_sem backpressure), so each
        # ft_sems[s] is quiescent between uses and its wait targets are
        # unambiguous even with many DMAs in flight. A single shared
        # counting semaphore would be racy: concurrent DMAs interleave
        # their 16 per-engine increments, so total>=16*(k+1) does not
        # prove DMA k completed.
        ft_sems = [
            ctx.enter_context(nc.semaphore(f"ft_sem{s}")) for s in range(FTB)
        ]
        dve_sem = ctx.enter_context(nc.semaphore("dve_sem"))
        pe_sem = ctx.enter_context(nc.semaphore("pe_sem"))
        act_sem = ctx.enter_context(nc.semaphore("act_sem"))
        out_sem = ctx.enter_context(nc.semaphore("out_sem"))
        xmt = ctx.enter_context(nc.sbuf_tensor("xmt", [P, KT * B + KT], _F32R))
        ftb = ctx.enter_context(nc.sbuf_tensor("ftb", [P, FTB * NSH], _F32R))
        ob = ctx.enter_context(nc.sbuf_tensor("ob", [B, NSH], _F32))
        ps = ctx.enter_context(nc.psum_tensor("ps", [B, NJ * 512], _F32))
        block = ctx.enter_context(nc.Block())

        def _stream(eng, parity):
            # Even k on the SP HWDGE ring, odd k on the Activation ring —
            # two descriptor pipelines in parallel.
            for k in range(parity, KT, 2):
                if k >= FTB:
                    # Ring slot k%FTB is free once tile k-FTB's matmuls ran.
                    eng.wait_ge(pe_sem, NJ * (k - FTB + 1))
                s = (k % FTB) * NSH
                eng.dma_start(
                    ftb[:, s:s + NSH], ft[k * P:(k + 1) * P, :]
                ).then_inc(ft_sems[k % FTB], 16)

        @block.sync
        def _(sync):
            sync.dma_start(xmt[:, :], xin[:, :]).then_inc(x_sem, 16)
            _stream(sync, 0)
            sync.wait_ge(act_sem, NJ)
            sync.dma_start(out[:, :], ob[:, :]).then_inc(out_sem, 16)
            sync.wait_ge(out_sem, 16)

        @block.vector
        def _(vector):
            vector.wait_ge(x_sem, 16)
            # x_masked^T in one DVE op: [128, 64, 16] * mask[128, 64, 1]
            xmt3 = xmt[:, :KT * B].rearrange("p (k b) -> p k b", b=B)
            vector.tensor_tensor(
                xmt3,
                xmt3,
                xmt[:, KT * B:][:, :, None].broadcast_to([P, KT, B]),
                mybir.AluOpType.mult,
            ).then_inc(dve_sem, 1)

        @block.tensor
        def _(tensor):
            tensor.wait_ge(dve_sem, 1)
            for k in range(KT):
                tensor.wait_ge(ft_sems[k % FTB], 16 * (k // FTB + 1))
                s = (k % FTB) * NSH
                for j in range(NJ):
                    # float32r: single-pass fp32 matmul (1 cycle/row at this
                    # moving size vs 4 for plain fp32) — keeps PE well under
                    # the DMA roofline. Exactness verified on HW: from_to is
                    # one-hot so every output is x*1.0 + zeros.
                    tensor.matmul(
                        ps[:, j * 512:(j + 1) * 512],
                        xmt[:, k * B:(k + 1) * B],
                        ftb[:, s + j * 512:s + (j + 1) * 512],
                        start=(k == 0),
                        stop=(k == KT - 1),
                    ).then_inc(pe_sem, 1)

        @block.scalar
        def _(scalar):
            _stream(scalar, 1)
            scalar.wait_ge(pe_sem, NJ * KT)
            for j in range(NJ):
                scalar.copy(
                    ob[:, j * 512:(j + 1) * 512], ps[:, j * 512:(j + 1) * 512]
                ).then_inc(act_sem, 1)

    return nc


def _prepare_in_maps(x, mask, from_to):
    x = np.asarray(x, dtype=np.float32)
    mask = np.asarray(mask, dtype=np.float32)
    from_to = np.asarray(from_to, dtype=np.float32)

    # [128, 64*16] with xt2[p, k*B+b] = x[b, k*128+p]
    xt2 = x.reshape(B, KT, P).transpose(2, 1, 0).reshape(P, KT * B)
    mk = mask.reshape(KT, P).T
    xin = np.ascontiguousarray(np.concatenate([xt2, mk], axis=1))

    in_maps = []
    for c in range(NCORES):
        ftc = np.ascontiguousarray(from_to[:, c * NSH:(c + 1) * NSH])
        in_maps.append({"xin": xin, "ft": ftc})
    return in_maps


def _run(x, mask, from_to, trace=False):
    nc = build_nc()
    in_maps = _prepare_in_maps(x, mask, from_to)
    res = run_bass_kernel_spmd(nc, in_maps, core_ids=list(range(NCORES)), trace=trace)
    out = np.concatenate([res.results[c]["out"] for c in range(NCORES)], axis=1)
    return out, res


def kernel(x, mask, from_to):
    out, _ = _run(x, mask, from_to, trace=False)
    return out


# revision 22
# speedup vs baseline: 1.1361x; 1.1361x over previous
"""Trainium2 Bass kernel for nn_Mask_58351425683882.

Computes out = (x * mask) @ from_to with
  x:      [16, 8192]  f32
  mask:   [8192]      f32 (0/1)
  from_to:[8192,8192] f32 (one-hot permutation columns)

Strategy: column-shard from_to across 8 NeuronCores ([8192, 1024] per
core), replicate x/mask. Each core streams its 32MB from_to shard from
HBM (the memory-roofline term) and accumulates the [16, 1024] output
slice on TensorE with x_masked^T as the stationary operand. Host
concatenates the 8 output slices.

Written in raw Bass (explicit engine blocks + semaphores): the Tile
scheduler attaches multi-semaphore waits to DMA/matmul instructions,
which this walrus build rejects ("Too many sync wait commands" — the
HWDGE/LW instruction encodings carry at most one). Raw standalone
wait_ge instructions sidestep that entirely.
"""

import sys

for _p in ("/opt/trn_rl_repo",):
    if _p not in sys.path:
        sys.path.insert(0, _p)

import numpy as np

import concourse.bass as bass
import concourse.mybir as mybir
from concourse.bass_utils import run_bass_kernel_spmd

B = 16          # batch rows of x
N = 8192        # feature dim
NCORES = 8
NSH = N // NCORES       # 1024 output columns per core
P = 128                 # SBUF partitions
KT = N // P             # 64 contraction tiles
NJ = NSH // 512         # 2 PSUM column chunks per core
FTB = 16                # ft streaming buffer depth (ring of SBUF slots)

_F32 = mybir.dt.float32
_F32R = mybir.dt.float32r


def build_nc():
    nc = bass.Bass()

    # xin packs x^T and mask:
    #   cols [0, KT*B):    xin[p, k*B + b] = x[b, k*128 + p]
    #   cols [KT*B, +KT):  xin[p, KT*B + k] = mask[k*128 + p]
    xin = nc.dram_tensor("xin", [P, KT * B + KT], _F32R, kind="ExternalInput")
    # This core's column shard of from_to.
    ft = nc.dram_tensor("ft", [N, NSH], _F32R, kind="ExternalInput")
    out = nc.dram_tensor("out", [B, NSH], _F32, kind="ExternalOutput")

    from contextlib import ExitStack

    with ExitStack() as ctx:
        x_sem = ctx.enter_context(nc.semaphore("x_sem"))
        # One semaphore per ring slot: slot s is reused only after the PE
        # consumed the previous tile in it (pe_sem backpressure), so each
        # ft_sems[s] is quiescent between uses and its wait targets are
        # unambiguous even with many DMAs in flight. A single shared
        # counting semaphore would be racy: concurrent DMAs interleave
        # their 16 per-engine increments, so total>=16*(k+1) does not
        # prove DMA k completed.
        ft_sems = [
            ctx.enter_context(nc.semaphore(f"ft_sem{s}")) for s in range(FTB)
        ]
        dve_sem = ctx.enter_context(nc.semaphore("dve_sem"))
        pe_sem = ctx.enter_context(nc.semaphore("pe_sem"))
        act_sem = ctx.enter_context(nc.semaphore("act_sem"))
        out_sem = ctx.enter_context(nc.semaphore("out_sem"))
        xmt = ctx.enter_context(nc.sbuf_tensor("xmt", [P, KT * B + KT], _F32R))
        ftb = ctx.enter_context(nc.sbuf_tensor("ftb", [P, FTB * NSH], _F32R))
        ob = ctx.enter_context(nc.sbuf_tensor("ob", [B, NSH], _F32))
        ps = ctx.enter_context(nc.psum_tensor("ps", [B, NJ * 512], _F32))
        block = ctx.enter_context(nc.Block())

        @block.sync
        def _(sync):
            sync.dma_start(xmt[:, :], xin[:, :]).then_inc(x_sem, 16)
            for k in range(KT):
                if k >= FTB:
                    # Ring slot k%FTB is free once tile k-FTB's matmuls ran.
                    sync.wait_ge(pe_sem, NJ * (k - FTB + 1))
                s = (k % FTB) * NSH
                sync.dma_start(
                    ftb[:, s:s + NSH], ft[k * P:(k + 1) * P, :]
                ).then_inc(ft_sems[k % FTB], 16)
            sync.wait_ge(act_sem, NJ)
            sync.dma_start(out[:, :], ob[:, :]).then_inc(out_sem, 16)
            sync.wait_ge(out_sem, 16)

        @block.vector
        def _(vector):
            vector.wait_ge(x_sem, 16)
            # x_masked^T in one DVE op: [128, 64, 16] * mask[128, 64, 1]
            xmt3 = xmt[:, :KT * B].rearrange("p (k b) -> p k b", b=B)
            vector.tensor_tensor(
                xmt3,
                xmt3,
                xmt[:, KT * B:][:, :, None].broadcast_to([P, KT, B]),
                mybir.AluOpType.mult,
            ).then_inc(dve_sem, 1)

        @block.tensor
        def _(tensor):
            tensor.wait_ge(dve_sem, 1)
            for k in range(KT):
                tensor.wait_ge(ft_sems[k % FTB], 16 * (k // FTB + 1))
                s = (k % FTB) * NSH
                for j in range(NJ):
                    # float32r: single-pass fp32 matmul (1 cycle/row at this
                    # moving size vs 4 for plain fp32) — keeps PE well under
                    # the DMA roofline. Exactness verified on HW: from_to is
                    # one-hot so every output is x*1.0 + zeros.
                    tensor.matmul(
                        ps[:, j * 512:(j + 1) * 512],
                        xmt[:, k * B:(k + 1) * B],
                        ftb[:, s + j * 512:s + (j + 1) * 512],
                        start=(k == 0),
                        stop=(k == KT - 1),
                    ).then_inc(pe_sem, 1)

        @block.scalar
        def _(scalar):
            scalar.wait_ge(pe_sem, NJ * KT)
            for j in range(NJ):
                scalar.copy(
                    ob[:, j * 512:(j + 1) * 512], ps[:, j * 512:(j + 1) * 512]
                ).then_inc(act_sem, 1)

    return nc


def _prepare_in_maps(x, mask, from_to):
    x = np.asarray(x, dtype=np.float32)
    mask = np.asarray(mask, dtype=np.float32)
    from_to = np.asarray(from_to, dtype=np.float32)

    # [128, 64*16] with xt2[p, k*B+b] = x[b, k*128+p]
    xt2 = x.reshape(B, KT, P).transpose(2, 1, 0).reshape(P, KT * B)
    mk = mask.reshape(KT, P).T
    xin = np.ascontiguousarray(np.concatenate([xt2, mk], axis=1))

    in_maps = []
    for c in range(NCORES):
        ftc = np.ascontiguousarray(from_to[:, c * NSH:(c + 1) * NSH])
        in_maps.append({"xin": xin, "ft": ftc})
    return in_maps


def _run(x, mask, from_to, trace=False):
    nc = build_nc()
    in_maps = _prepare_in_maps(x, mask, from_to)
    res = run_bass_kernel_spmd(nc, in_maps, core_ids=list(range(NCORES)), trace=trace)
    out = np.concatenate([res.results[c]["out"] for c in range(NCORES)], axis=1)
    return out, res


def kernel(x, mask, from_to):
    out, _ = _run(x, mask, from_to, trace=False)
    return out
